# revision 10
# baseline (speedup 1.0000x reference)
"""Trainium2 Bass kernel for DreamerV2-style ImagBehavior imagination rollout
(vq_codebook): 15-step scan of [VQ-quantize -> lam RSSM step -> actor ->
dyn RSSM step] over B=2048 rows, data-parallel over 8 NeuronCores
(256 rows/core), fp32 matmuls (exact), feature-major activation layout.
"""
import sys

sys.path.insert(0, '/opt/trn_rl_repo')

import numpy as np

import concourse.bass as bass
import concourse.tile as tile
from concourse import bacc, mybir
from concourse import bass_utils
from concourse.masks import make_identity

dt = mybir.dt
AF = mybir.ActivationFunctionType
OP = mybir.AluOpType

H = 15
B = 2048
NCORES = 8
BC = B // NCORES          # 256 rows per core
NB = BC // 128            # 2 batch tiles of 128
STOCH, DISC = 32, 32
SFLAT = 1024
DETER = 1024
FEAT = 2048
LAT = 128
CODES = 512
ACT_DIM = 12
UNITS = 512
HID = 1024

_cached = {}


def _f32(x):
    return np.ascontiguousarray(np.asarray(x), dtype=np.float32)


def build(h_steps=H):
    nc = bacc.Bacc("TRN2", target_bir_lowering=False, debug=False)

    def din(name, shape):
        return nc.dram_tensor(name, list(shape), dt.float32, kind="ExternalInput")

    def dout(name, shape):
        return nc.dram_tensor(name, list(shape), dt.float32, kind="ExternalOutput")

    # --- inputs (host pre-tiled layouts; see kernel() for the packing) ---
    SSTOCHT = din("sstocht", [128, 8 * BC])     # start_stoch^T tiled
    SDETERT = din("sdetert", [128, 8 * BC])
    SSTOCH = din("sstoch", [BC, SFLAT])         # batch-major shard
    SDETER = din("sdeter", [BC, DETER])
    WIM = din("wim", [128, 16 * 128])
    CBT = din("cbt", [128, CODES])              # codebook^T
    CB = din("cb", [128, 4 * LAT])              # codebook k-tiled (lhsT for quant)
    CNORM = din("cnorm", [128, CODES])          # |c|^2 broadcast over partitions
    WIN = {p: din(f"win_{p}", [128, 8 * 9 * 128]) for p in ("lam", "dyn")}
    WGRU = {p: din(f"wgru_{p}", [128, 8 * 16 * 3 * 128]) for p in ("lam", "dyn")}
    WOUT = {p: din(f"wout_{p}", [128, 8 * 8 * 128]) for p in ("lam", "dyn")}
    WLOGIT = {p: din(f"wlogit_{p}", [128, 2 * 8 * 512]) for p in ("lam", "dyn")}
    W1 = din("w1", [128, 4 * 17 * 128])
    W2 = din("w2", [128, 4 * 4 * 128])
    W3 = din("w3", [128, 4 * ACT_DIM])
    BIM = din("bim", [128, 1])
    BIN = {p: din(f"bin_{p}", [128, 8]) for p in ("lam", "dyn")}
    BGRU = {p: din(f"bgru_{p}", [128, 24]) for p in ("lam", "dyn")}
    BOUT = {p: din(f"bout_{p}", [128, 8]) for p in ("lam", "dyn")}
    BLOGIT = {p: din(f"blogit_{p}", [128, SFLAT]) for p in ("lam", "dyn")}
    B1 = din("b1", [128, 4])
    B2 = din("b2", [128, 4])
    B3 = din("b3", [ACT_DIM, 1])

    ACTIONS = dout("actions", [h_steps, BC, ACT_DIM])
    OSTOCH = dout("ostoch", [h_steps, BC, SFLAT])
    ODETER = dout("odeter", [h_steps, BC, DETER])
    FQ = dout("fq", [h_steps, BC, LAT])         # quant part of feats
    FFS = dout("ffs", [h_steps, BC, SFLAT])     # feat_future stoch part
    FFD = dout("ffd", [h_steps, BC, DETER])     # feat_future deter part

    with tile.TileContext(nc) as tc:
        with (
            tc.tile_pool(name="res", bufs=1) as res,        # resident constants
            tc.tile_pool(name="carry", bufs=2) as carry,    # stochT/deterT ping-pong
            tc.tile_pool(name="acts", bufs=1) as acts,      # per-step activation tensors
            tc.tile_pool(name="tmp", bufs=2) as tmp,        # small elementwise temps
            tc.tile_pool(name="wstream", bufs=1) as wst,    # streamed weights (bufs via tile())
            tc.tile_pool(name="pmm", bufs=4, space="PSUM") as pmm,
            tc.tile_pool(name="pwide", bufs=2, space="PSUM") as pwide,
            tc.tile_pool(name="ptr", bufs=2, space="PSUM") as ptr,
        ):
            # ---------------- setup: resident tiles ----------------
            ident = res.tile([128, 128], dt.float32, name="ident")
            make_identity(nc, ident[:])

            def load_res(name, dram, shape):
                tl = res.tile(list(shape), dt.float32, name=name)
                nc.sync.dma_start(tl[:], dram[tuple(slice(0, s) for s in shape)])
                return tl

            wim = load_res("wim_t", WIM, [128, 2048])
            cbt = load_res("cbt_t", CBT, [128, CODES])
            cb = load_res("cb_t", CB, [128, 512])
            cnorm = load_res("cnorm_t", CNORM, [128, CODES])
            w2 = load_res("w2_t", W2, [128, 2048])
            w3 = load_res("w3_t", W3, [128, 48])
            bim = load_res("bim_t", BIM, [128, 1])
            b1 = load_res("b1_t", B1, [128, 4])
            b2 = load_res("b2_t", B2, [128, 4])
            b3 = load_res("b3_t", B3, [ACT_DIM, 1])
            bin_t = {p: load_res(f"bin_t{p}", BIN[p], [128, 8]) for p in ("lam", "dyn")}
            bgru_t = {p: load_res(f"bgru_t{p}", BGRU[p], [128, 24]) for p in ("lam", "dyn")}
            bout_t = {p: load_res(f"bout_t{p}", BOUT[p], [128, 8]) for p in ("lam", "dyn")}
            blogit_t = {p: load_res(f"blogit_t{p}", BLOGIT[p], [128, SFLAT]) for p in ("lam", "dyn")}

            # initial carry
            stochT = carry.tile([128, 8 * BC], dt.float32, name="stochT0", tag="stochT")
            deterT = carry.tile([128, 8 * BC], dt.float32, name="deterT0", tag="deterT")
            nc.sync.dma_start(stochT[:], SSTOCHT[:, :])
            nc.sync.dma_start(deterT[:], SDETERT[:, :])

            # t=0 state outputs straight from DRAM inputs
            nc.sync.dma_start(OSTOCH[0, :, :], SSTOCH[:, :])
            nc.sync.dma_start(ODETER[0, :, :], SDETER[:, :])

            # rhs block helpers: feature-major tensors are [128, nblk*BC] tiles,
            # block k at cols [k*BC, (k+1)*BC)
            def blk(tl, k):
                return tl[:, k * BC:(k + 1) * BC]

            def elu1(dst_tile, n, psum, bias_col, t, p, nm):
                """dst block n <- elu(psum + bias) + 1 = relu(x+b) + exp(min(x+b,0))"""
                m = tmp.tile([128, BC], dt.float32, name=f"m{nm}", tag="elu_m")
                nc.vector.tensor_scalar(m[:], psum[:], bias_col, 0.0, OP.add, OP.min)
                e = tmp.tile([128, BC], dt.float32, name=f"e{nm}", tag="elu_e")
                nc.scalar.activation(e[:], m[:], AF.Exp)
                r = tmp.tile([128, BC], dt.float32, name=f"r{nm}", tag="elu_r")
                nc.scalar.activation(r[:], psum[:], AF.Relu, bias=bias_col)
                nc.vector.tensor_tensor(blk(dst_tile, n), r[:], e[:], OP.add)

            def transpose_to(dst_ap, src_ap, nm, use_act, kdim=128):
                """dst_ap (SBUF) <- transpose of src_ap [kdim,128] via PE + copy"""
                pt = ptr.tile([128, 128], dt.float32, name=f"pt{nm}", tag="ptr")
                nc.tensor.transpose(pt[0:src_ap.shape[1], 0:kdim], src_ap, ident[0:kdim, 0:kdim])
                if use_act:
                    nc.scalar.activation(dst_ap, pt[0:src_ap.shape[1], 0:kdim], AF.Copy)
                else:
                    nc.vector.tensor_copy(dst_ap, pt[0:src_ap.shape[1], 0:kdim])

            # ---------------- the imagination steps ----------------
            for t in range(h_steps):
                nm_t = f"_{t}"

                # ---- z = feat @ W_im + b_im   (feature-major, K=16 tiles) ----
                pz = pmm.tile([128, BC], dt.float32, name=f"pz{nm_t}", tag="pmm")
                for k in range(16):
                    rhs = blk(stochT, k) if k < 8 else blk(deterT, k - 8)
                    nc.tensor.matmul(pz[:], wim[:, k * 128:(k + 1) * 128], rhs,
                                     start=(k == 0), stop=(k == 15))
                zT = acts.tile([128, BC], dt.float32, name=f"zT{nm_t}", tag="zT")
                nc.vector.tensor_scalar(zT[:], pz[:], bim[:], None, OP.add)

                # ---- VQ: dist per batch-tile, argmin one-hot, transpose ----
                oh512T = acts.tile([128, 4 * BC], dt.float32, name=f"oh512T{nm_t}", tag="oh512T")
                for i in range(NB):
                    pd = pwide.tile([128, CODES], dt.float32, name=f"pd{nm_t}_{i}", tag="pwide")
                    nc.tensor.matmul(pd[:], zT[:, i * 128:(i + 1) * 128], cbt[:],
                                     start=True, stop=True)
                    dist = tmp.tile([128, CODES], dt.float32, name=f"dist{nm_t}_{i}", tag="dist")
                    nc.vector.scalar_tensor_tensor(dist[:], pd[:], -2.0, cnorm[:], OP.mult, OP.add)
                    dmin = tmp.tile([128, 1], dt.float32, name=f"dmin{nm_t}_{i}", tag="dmin")
                    nc.vector.tensor_reduce(dmin[:], dist[:], mybir.AxisListType.X, OP.min)
                    oh = tmp.tile([128, CODES], dt.float32, name=f"oh5{nm_t}_{i}", tag="oh512b")
                    nc.vector.tensor_scalar(oh[:], dist[:], dmin[:], None, OP.is_equal)
                    for c in range(4):
                        transpose_to(oh512T[:, c * BC + i * 128: c * BC + (i + 1) * 128],
                                     oh[:, c * 128:(c + 1) * 128],
                                     f"o{nm_t}_{i}_{c}", use_act=(c % 2 == 0))

                # quantT = codebook^T @ onehot  (K = 4 code tiles)
                pq = pmm.tile([128, BC], dt.float32, name=f"pq{nm_t}", tag="pmm")
                for c in range(4):
                    nc.tensor.matmul(pq[:], cb[:, c * 128:(c + 1) * 128],
                                     oh512T[:, c * BC:(c + 1) * BC],
                                     start=(c == 0), stop=(c == 3))
                quantT = acts.tile([128, BC], dt.float32, name=f"quantT{nm_t}", tag="quantT")
                nc.vector.tensor_copy(quantT[:], pq[:])

                # quant batch-major -> feats[t][:, 2048:2176]
                for i in range(NB):
                    qb = tmp.tile([128, LAT], dt.float32, name=f"qb{nm_t}_{i}", tag="qb")
                    transpose_to(qb[:], quantT[:, i * 128:(i + 1) * 128], f"qb{nm_t}_{i}",
                                 use_act=(i == 0))
                    nc.sync.dma_start(FQ[t, i * 128:(i + 1) * 128, :], qb[:])

                # ---- RSSM branch (shared for lam / dyn) ----
                def branch(p, act_tile):
                    nm = f"{nm_t}{p}"
                    # x_in = elu(concat(stoch, act) @ Win + bin) + 1
                    vT = acts.tile([128, 8 * BC], dt.float32, name=f"vT{nm}", tag="big8", bufs=2)
                    for n in range(8):
                        wch = wst.tile([128, 9 * 128], dt.float32, name=f"win{nm}_{n}",
                                       tag="win", bufs=2)
                        nc.sync.dma_start(wch[:], WIN[p][:, n * 1152:(n + 1) * 1152])
                        px = pmm.tile([128, BC], dt.float32, name=f"px{nm}_{n}", tag="pmm")
                        for k in range(9):
                            rhs = blk(stochT, k) if k < 8 else act_tile[:, 0:BC]
                            nc.tensor.matmul(px[:], wch[:, k * 128:(k + 1) * 128], rhs,
                                             start=(k == 0), stop=(k == 8))
                        elu1(vT, n, px, bin_t[p][:, n:n + 1], t, p, f"x{nm}_{n}")

                    # GRU: parts = concat(v, deter) @ Wgru + bgru_adj
                    if p == "lam":
                        ndet = acts.tile([128, 8 * BC], dt.float32, name=f"fdet{nm}", tag="big8", bufs=2)
                    else:
                        ndet = carry.tile([128, 8 * BC], dt.float32, name=f"ndet{nm}", tag="deterT")
                    for f in range(8):
                        pr = pmm.tile([128, BC], dt.float32, name=f"pr{nm}_{f}", tag="pmm")
                        pc = pmm.tile([128, BC], dt.float32, name=f"pc{nm}_{f}", tag="pmm")
                        pu = pmm.tile([128, BC], dt.float32, name=f"pu{nm}_{f}", tag="pmm")
                        gps = [pr, pc, pu]
                        for q in range(4):
                            gch = wst.tile([128, 4 * 384], dt.float32, name=f"wgru{nm}_{f}_{q}",
                                           tag="wgru", bufs=2)
                            nc.sync.dma_start(gch[:], WGRU[p][:, f * 6144 + q * 1536:
                                                              f * 6144 + (q + 1) * 1536])
                            for kk in range(4):
                                k = q * 4 + kk
                                rhs = blk(vT, k) if k < 8 else blk(deterT, k - 8)
                                for g in range(3):
                                    nc.tensor.matmul(gps[g][:],
                                                     gch[:, kk * 384 + g * 128: kk * 384 + (g + 1) * 128],
                                                     rhs, start=(k == 0), stop=(k == 15))
                        bg = bgru_t[p]
                        r_sb = tmp.tile([128, BC], dt.float32, name=f"gr{nm}_{f}", tag="g_r")
                        nc.scalar.activation(r_sb[:], pr[:], AF.Sigmoid, bias=bg[:, f:f + 1])
                        t_sb = tmp.tile([128, BC], dt.float32, name=f"gt{nm}_{f}", tag="g_t")
                        nc.vector.scalar_tensor_tensor(t_sb[:], pc[:], bg[:, 8 + f:9 + f], r_sb[:],
                                                       OP.add, OP.mult)
                        c_sb = tmp.tile([128, BC], dt.float32, name=f"gc{nm}_{f}", tag="g_c")
                        nc.scalar.activation(c_sb[:], t_sb[:], AF.Tanh)
                        u_sb = tmp.tile([128, BC], dt.float32, name=f"gu{nm}_{f}", tag="g_u")
                        nc.scalar.activation(u_sb[:], pu[:], AF.Sigmoid, bias=bg[:, 16 + f:17 + f])
                        s_sb = tmp.tile([128, BC], dt.float32, name=f"gs{nm}_{f}", tag="g_s")
                        nc.vector.tensor_tensor(s_sb[:], c_sb[:], blk(deterT, f), OP.subtract)
                        w_sb = tmp.tile([128, BC], dt.float32, name=f"gw{nm}_{f}", tag="g_w")
                        nc.vector.tensor_tensor(w_sb[:], s_sb[:], u_sb[:], OP.mult)
                        nc.vector.tensor_tensor(blk(ndet, f), w_sb[:], blk(deterT, f), OP.add)

                    # y' = elu(ndet @ Wout + bout) + 1
                    yT = acts.tile([128, 8 * BC], dt.float32, name=f"yT{nm}", tag="big8", bufs=2)
                    for n in range(8):
                        och = wst.tile([128, 8 * 128], dt.float32, name=f"wout{nm}_{n}",
                                       tag="wout", bufs=2)
                        nc.sync.dma_start(och[:], WOUT[p][:, n * 1024:(n + 1) * 1024])
                        py = pmm.tile([128, BC], dt.float32, name=f"py{nm}_{n}", tag="pmm")
                        for k in range(8):
                            nc.tensor.matmul(py[:], och[:, k * 128:(k + 1) * 128], blk(ndet, k),
                                             start=(k == 0), stop=(k == 7))
                        elu1(yT, n, py, bout_t[p][:, n:n + 1], t, p, f"y{nm}_{n}")

                    # logits (batch-major, option A): lhsT = yT slices, rhs = Wlogit
                    lg = [acts.tile([128, SFLAT], dt.float32, name=f"lg{nm}_{i}", tag=f"lg{i}")
                          for i in range(NB)]
                    for hh in range(2):
                        pl = [pwide.tile([128, 512], dt.float32, name=f"pl{nm}_{hh}_{i}",
                                         tag="pwide") for i in range(NB)]
                        for k2 in range(4):
                            lch = wst.tile([128, 2 * 512], dt.float32, name=f"wlog{nm}_{hh}_{k2}",
                                           tag="wlogit", bufs=2)
                            nc.sync.dma_start(lch[:], WLOGIT[p][:, hh * 4096 + k2 * 1024:
                                                                hh * 4096 + (k2 + 1) * 1024])
                            for kk in range(2):
                                k = k2 * 2 + kk
                                for i in range(NB):
                                    nc.tensor.matmul(pl[i][:],
                                                     yT[:, k * BC + i * 128: k * BC + (i + 1) * 128],
                                                     lch[:, kk * 512:(kk + 1) * 512],
                                                     start=(k == 0), stop=(k == 7))
                        for i in range(NB):
                            nc.vector.tensor_tensor(lg[i][:, hh * 512:(hh + 1) * 512], pl[i][:],
                                                    blogit_t[p][:, hh * 512:(hh + 1) * 512], OP.add)
                    ohb = []
                    for i in range(NB):
                        mx = tmp.tile([128, STOCH], dt.float32, name=f"mx{nm}_{i}", tag="mx")
                        lg3 = lg[i][:].rearrange("p (g d) -> p g d", d=DISC)
                        nc.vector.tensor_reduce(mx[:], lg3, mybir.AxisListType.X, OP.max)
                        ohi = acts.tile([128, SFLAT], dt.float32, name=f"ohb{nm}_{i}",
                                        tag=f"ohb_{i}")
                        nc.vector.tensor_tensor(ohi[:].rearrange("p (g d) -> p g d", d=DISC), lg3,
                                                mx[:, :, None].broadcast_to([128, STOCH, DISC]),
                                                OP.is_equal)
                        ohb.append(ohi)
                    return vT, ndet, yT, ohb

                # ======== LAM branch -> feat_future[t] ========
                vT_l, fdet, yT_l, ohb_l = branch("lam", quantT)
                for i in range(NB):
                    nc.sync.dma_start(FFS[t, i * 128:(i + 1) * 128, :], ohb_l[i][:])
                for i in range(NB):
                    db = tmp.tile([128, DETER], dt.float32, name=f"dbl{nm_t}_{i}", tag=f"db{i}", bufs=1)
                    for f in range(8):
                        transpose_to(db[:, f * 128:(f + 1) * 128],
                                     fdet[:, f * BC + i * 128: f * BC + (i + 1) * 128],
                                     f"dl{nm_t}_{i}_{f}", use_act=(f % 2 == 0))
                    nc.sync.dma_start(FFD[t, i * 128:(i + 1) * 128, :], db[:])

                # ======== actor: action = tanh(MLP(concat(feat_ori, quant))) ========
                h1T = acts.tile([128, 4 * BC], dt.float32, name=f"h1T{nm_t}", tag="h1T")
                for n in range(4):
                    wch = wst.tile([128, 17 * 128], dt.float32, name=f"w1{nm_t}_{n}",
                                   tag="w1", bufs=1)
                    nc.sync.dma_start(wch[:], W1[:, n * 2176:(n + 1) * 2176])
                    ph = pmm.tile([128, BC], dt.float32, name=f"ph1{nm_t}_{n}", tag="pmm")
                    for k in range(17):
                        rhs = blk(stochT, k) if k < 8 else (
                            blk(deterT, k - 8) if k < 16 else quantT[:, 0:BC])
                        nc.tensor.matmul(ph[:], wch[:, k * 128:(k + 1) * 128], rhs,
                                         start=(k == 0), stop=(k == 16))
                    elu1(h1T, n, ph, b1[:, n:n + 1], t, "a", f"h1{nm_t}_{n}")
                h2T = acts.tile([128, 4 * BC], dt.float32, name=f"h2T{nm_t}", tag="h2T")
                for n in range(4):
                    ph = pmm.tile([128, BC], dt.float32, name=f"ph2{nm_t}_{n}", tag="pmm")
                    for k in range(4):
                        nc.tensor.matmul(ph[:], w2[:, n * 512 + k * 128: n * 512 + (k + 1) * 128],
                                         blk(h1T, k), start=(k == 0), stop=(k == 3))
                    elu1(h2T, n, ph, b2[:, n:n + 1], t, "a", f"h2{nm_t}_{n}")
                pa = pmm.tile([ACT_DIM, BC], dt.float32, name=f"pa{nm_t}", tag="pmm")
                for k in range(4):
                    nc.tensor.matmul(pa[:], w3[:, k * ACT_DIM:(k + 1) * ACT_DIM], blk(h2T, k),
                                     start=(k == 0), stop=(k == 3))
                actT = acts.tile([128, BC], dt.float32, name=f"actT{nm_t}", tag="actT")
                nc.gpsimd.memset(actT[:], 0.0)
                nc.scalar.activation(actT[0:ACT_DIM, :], pa[:], AF.Tanh, bias=b3[:])
                for i in range(NB):
                    ab = tmp.tile([128, ACT_DIM], dt.float32, name=f"ab{nm_t}_{i}", tag="ab")
                    transpose_to(ab[:], actT[0:ACT_DIM, i * 128:(i + 1) * 128],
                                 f"ab{nm_t}_{i}", use_act=(i == 0), kdim=ACT_DIM)
                    nc.sync.dma_start(ACTIONS[t, i * 128:(i + 1) * 128, :], ab[:])

                # ======== DYN branch -> next state (skipped at final step) ========
                if t < h_steps - 1:
                    vT_d, ndet, yT_d, ohb_d = branch("dyn", actT)
                    # next-state batch-major outputs
                    for i in range(NB):
                        sl = slice(i * 128, (i + 1) * 128)
                        nc.sync.dma_start(OSTOCH[t + 1, sl, :], ohb_d[i][:])
                        db = tmp.tile([128, DETER], dt.float32, name=f"dbd{nm_t}_{i}", tag=f"db{i}", bufs=1)
                        for f in range(8):
                            transpose_to(db[:, f * 128:(f + 1) * 128],
                                         ndet[:, f * BC + i * 128: f * BC + (i + 1) * 128],
                                         f"dd{nm_t}_{i}_{f}", use_act=(f % 2 == 1))
                        nc.sync.dma_start(ODETER[t + 1, sl, :], db[:])
                    # next stochT (feature-major) from one-hots
                    nstoch = carry.tile([128, 8 * BC], dt.float32, name=f"nst{nm_t}", tag="stochT")
                    for f in range(8):
                        for i in range(NB):
                            transpose_to(nstoch[:, f * BC + i * 128: f * BC + (i + 1) * 128],
                                         ohb_d[i][:, f * 128:(f + 1) * 128],
                                         f"st{nm_t}_{i}_{f}", use_act=((f + i) % 2 == 0))
                    stochT, deterT = nstoch, ndet

    nc.compile()
    names = dict(
        outs=["actions", "ostoch", "odeter", "fq", "ffs", "ffd"],
    )
    return nc, names


def _prep_shared(d):
    """Host-side packing of weights/biases shared by all cores."""
    f = _f32
    w_im = f(d["W_im"]); b_im = f(d["b_im"]); cbk = f(d["codebook"])
    out = {}
    out["wim"] = w_im.reshape(16, 128, 128).transpose(1, 0, 2).reshape(128, 2048).copy()
    out["cbt"] = cbk.T.copy()
    out["cb"] = cbk.reshape(4, 128, 128).transpose(1, 0, 2).reshape(128, 512).copy()
    out["cnorm"] = np.tile((cbk * cbk).sum(1)[None, :], (128, 1)).astype(np.float32)
    out["bim"] = b_im[:, None].copy()
    for p, adim in (("lam", LAT), ("dyn", ACT_DIM)):
        win = f(d[p + "_Win"]); bin_ = f(d[p + "_bin"])
        wgru = f(d[p + "_Wgru"]); bgru = f(d[p + "_bgru"])
        wout = f(d[p + "_Wout"]); bout = f(d[p + "_bout"])
        wlog = f(d[p + "_Wlogit"]); blog = f(d[p + "_blogit"])
        win_pad = np.zeros((1152, 1024), np.float32)
        win_pad[: win.shape[0]] = win
        out[f"win_{p}"] = (win_pad.reshape(9, 128, 8, 128)
                           .transpose(2, 0, 1, 3)    # [n, k, p, c]
                           .transpose(2, 0, 1, 3)    # [p, n, k, c]
                           .reshape(128, 8 * 9 * 128).copy())
        bgru_adj = bgru - wgru[:1024].sum(0)
        bgru_adj = bgru_adj.copy()
        bgru_adj[2048:] -= 1.0    # update gate's  sigmoid(u - 1)
        out[f"wgru_{p}"] = (wgru.reshape(16, 128, 3, 8, 128)
                            .transpose(1, 3, 0, 2, 4)   # [p, f, k, g, c]
                            .reshape(128, 8 * 16 * 3 * 128).copy())
        out[f"wout_{p}"] = (wout.reshape(8, 128, 8, 128)
                            .transpose(1, 2, 0, 3)      # [p, n, k, c]
                            .reshape(128, 8 * 8 * 128).copy())
        out[f"wlogit_{p}"] = (wlog.reshape(8, 128, 2, 512)
                              .transpose(1, 2, 0, 3)    # [p, h, k, c]
                              .reshape(128, 2 * 8 * 512).copy())
        blog_adj = blog - wlog.sum(0)
        out[f"bin_{p}"] = bin_.reshape(8, 128).T.copy()
        out[f"bgru_{p}"] = bgru_adj.reshape(3, 8, 128).transpose(2, 0, 1).reshape(128, 24).copy()
        out[f"bout_{p}"] = bout.reshape(8, 128).T.copy()
        out[f"blogit_{p}"] = np.tile(blog_adj[None, :], (128, 1)).astype(np.float32)
    w1 = f(d["act_W1"]); w2_ = f(d["act_W2"]); w3_ = f(d["act_W3"])
    out["w1"] = (w1.reshape(17, 128, 4, 128).transpose(1, 2, 0, 3)
                 .reshape(128, 4 * 17 * 128).copy())
    out["w2"] = (w2_.reshape(4, 128, 4, 128).transpose(1, 2, 0, 3)
                 .reshape(128, 4 * 4 * 128).copy())
    out["w3"] = w3_.reshape(4, 128, ACT_DIM).transpose(1, 0, 2).reshape(128, 48).copy()
    out["b1"] = f(d["act_b1"]).reshape(4, 128).T.copy()
    out["b2"] = (f(d["act_b2"]) - w2_.sum(0)).reshape(4, 128).T.copy()
    out["b3"] = (f(d["act_b3"]) - w3_.sum(0))[:, None].copy()
    return out


def _prep_core(d, core):
    rows = slice(core * BC, (core + 1) * BC)
    ss = _f32(d["start_stoch"])[rows]
    sd = _f32(d["start_deter"])[rows]
    return {
        "sstocht": ss.T.reshape(8, 128, BC).transpose(1, 0, 2).reshape(128, 8 * BC).copy(),
        "sdetert": sd.T.reshape(8, 128, BC).transpose(1, 0, 2).reshape(128, 8 * BC).copy(),
        "sstoch": ss.copy(),
        "sdeter": sd.copy(),
    }


def _in_maps(inputs):
    shared = _prep_shared(inputs)
    return [dict(shared, **_prep_core(inputs, core)) for core in range(NCORES)]


def _assemble(parts):
    """parts: dict name -> full [H, B, D] array; returns reference-order tuple."""
    feat_ori = np.concatenate([parts["ostoch"], parts["odeter"]], axis=-1)
    feats = np.concatenate([feat_ori, parts["fq"]], axis=-1)
    feat_future = np.concatenate([parts["ffs"], parts["ffd"]], axis=-1)
    return (feats, parts["actions"], parts["ostoch"], parts["odeter"],
            feat_ori, feat_future)


def kernel(h_steps=H, **inputs):
    key = h_steps
    if key not in _cached:
        _cached[key] = build(h_steps)
    nc, names = _cached[key]
    res = bass_utils.run_bass_kernel_spmd(nc, _in_maps(inputs),
                                          core_ids=list(range(NCORES)))
    parts = {
        nm: np.concatenate([res.results[c][nm] for c in range(NCORES)], axis=1)
        for nm in names["outs"]
    }
    return _assemble(parts)


def timed_run(inputs, h_steps=H, iters=3):
    """Run on 8 cores with device-staged inputs; return (outputs, best wall ns)."""
    import time
    import jax
    from jax.experimental.shard_map import shard_map
    from jax.sharding import Mesh, NamedSharding, PartitionSpec
    from concourse import bass2jax, mybir as _mybir

    key = h_steps
    if key not in _cached:
        _cached[key] = build(h_steps)
    nc, names = _cached[key]
    bass2jax.install_neuronx_cc_hook()
    in_maps = _in_maps(inputs)

    partition_name = nc.partition_id_tensor.name if nc.partition_id_tensor else None
    in_names, out_names, out_avals, zero_outs = [], [], [], []
    for alloc in nc.m.functions[0].allocations:
        if not isinstance(alloc, _mybir.MemoryLocationSet):
            continue
        name = alloc.memorylocations[0].name
        if alloc.kind == "ExternalInput":
            if name != partition_name:
                in_names.append(name)
        elif alloc.kind == "ExternalOutput":
            shape = tuple(alloc.tensor_shape)
            np_dt = _mybir.dt.np(alloc.dtype)
            out_names.append(name)
            out_avals.append(jax.core.ShapedArray(shape, np_dt))
            zero_outs.append(np.zeros(shape, np_dt))
    n_params, n_outs = len(in_names), len(out_names)
    all_in = in_names + out_names + ([partition_name] if partition_name else [])

    def _body(*args):
        operands = list(args)
        if partition_name is not None:
            operands.append(bass2jax.partition_id_tensor())
        return tuple(bass2jax._bass_exec_p.bind(
            *operands, out_avals=tuple(out_avals), in_names=tuple(all_in),
            out_names=tuple(out_names), lowering_input_output_aliases=(),
            sim_require_finite=True, sim_require_nnan=True, nc=nc))

    devices = jax.devices()[:NCORES]
    mesh = Mesh(np.asarray(devices), ("core",))
    sharded = jax.jit(
        shard_map(_body, mesh=mesh, in_specs=(PartitionSpec("core"),) * (n_params + n_outs),
                  out_specs=(PartitionSpec("core"),) * n_outs, check_rep=False),
        donate_argnums=tuple(range(n_params, n_params + n_outs)), keep_unused=True)

    sh = NamedSharding(mesh, PartitionSpec("core"))
    concat_in = [
        jax.device_put(np.concatenate([np.asarray(in_maps[c][nm]) for c in range(NCORES)], 0), sh)
        for nm in in_names
    ]
    best = None
    out_arrs = None
    for _ in range(iters):
        zeros = [jax.device_put(np.zeros((NCORES * z.shape[0], *z.shape[1:]), z.dtype), sh)
                 for z in zero_outs]
        jax.block_until_ready(zeros)
        jax.block_until_ready(concat_in)
        t0 = time.perf_counter()
        out_arrs = sharded(*concat_in, *zeros)
        jax.block_until_ready(out_arrs)
        dtns = (time.perf_counter() - t0) * 1e9
        best = dtns if best is None else min(best, dtns)
    parts = {}
    for nm in names["outs"]:
        i = out_names.index(nm)
        a = np.asarray(out_arrs[i]).reshape(NCORES, *out_avals[i].shape)
        parts[nm] = np.concatenate([a[c] for c in range(NCORES)], axis=1)
    return _assemble(parts), best


# revision 11
# speedup vs baseline: 1.7965x; 1.7965x over previous
"""Trainium2 Bass kernel for DreamerV2-style ImagBehavior imagination rollout
(vq_codebook): 15-step scan of [VQ-quantize -> lam RSSM step -> actor ->
dyn RSSM step] over B=2048 rows, data-parallel over 8 NeuronCores
(256 rows/core), fp32 matmuls (exact), feature-major activation layout.
"""
import sys

sys.path.insert(0, '/opt/trn_rl_repo')

import numpy as np

import concourse.bass as bass
import concourse.tile as tile
from concourse import bacc, mybir
from concourse import bass_utils
from concourse.masks import make_identity

dt = mybir.dt
AF = mybir.ActivationFunctionType
OP = mybir.AluOpType

H = 15
B = 2048
NCORES = 8
BC = B // NCORES          # 256 rows per core
NB = BC // 128            # 2 batch tiles of 128
STOCH, DISC = 32, 32
SFLAT = 1024
DETER = 1024
FEAT = 2048
LAT = 128
CODES = 512
ACT_DIM = 12
UNITS = 512
HID = 1024

_cached = {}


def _f32(x):
    return np.ascontiguousarray(np.asarray(x), dtype=np.float32)


def build(h_steps=H):
    nc = bacc.Bacc("TRN2", target_bir_lowering=False, debug=False)

    def din(name, shape):
        return nc.dram_tensor(name, list(shape), dt.float32, kind="ExternalInput")

    def dout(name, shape):
        return nc.dram_tensor(name, list(shape), dt.float32, kind="ExternalOutput")

    # --- inputs (host pre-tiled layouts; see kernel() for the packing) ---
    SSTOCHT = din("sstocht", [128, 8 * BC])     # start_stoch^T tiled
    SDETERT = din("sdetert", [128, 8 * BC])
    SSTOCH = din("sstoch", [BC, SFLAT])         # batch-major shard
    SDETER = din("sdeter", [BC, DETER])
    WIM = din("wim", [128, 16 * 128])
    CBT = din("cbt", [128, CODES])              # codebook^T
    CB = din("cb", [128, 4 * LAT])              # codebook k-tiled (lhsT for quant)
    CNORM = din("cnorm", [128, CODES])          # |c|^2 broadcast over partitions
    WIN = {p: din(f"win_{p}", [128, 8 * 9 * 128]) for p in ("lam", "dyn")}
    WGRU = {p: din(f"wgru_{p}", [128, 8 * 16 * 3 * 128]) for p in ("lam", "dyn")}
    WOUT = {p: din(f"wout_{p}", [128, 8 * 8 * 128]) for p in ("lam", "dyn")}
    WLOGIT = {p: din(f"wlogit_{p}", [128, 2 * 8 * 512]) for p in ("lam", "dyn")}
    W1 = din("w1", [128, 4 * 17 * 128])
    W2 = din("w2", [128, 4 * 4 * 128])
    W3 = din("w3", [128, 4 * ACT_DIM])
    BIM = din("bim", [128, 1])
    BIN = {p: din(f"bin_{p}", [128, 8]) for p in ("lam", "dyn")}
    BGRU = {p: din(f"bgru_{p}", [128, 24]) for p in ("lam", "dyn")}
    BOUT = {p: din(f"bout_{p}", [128, 8]) for p in ("lam", "dyn")}
    BLOGIT = {p: din(f"blogit_{p}", [128, SFLAT]) for p in ("lam", "dyn")}
    B1 = din("b1", [128, 4])
    B2 = din("b2", [128, 4])
    B3 = din("b3", [ACT_DIM, 1])

    ACTIONS = dout("actions", [h_steps, BC, ACT_DIM])
    OSTOCH = dout("ostoch", [h_steps, BC, SFLAT])
    ODETER = dout("odeter", [h_steps, BC, DETER])
    FQ = dout("fq", [h_steps, BC, LAT])         # quant part of feats
    FFS = dout("ffs", [h_steps, BC, SFLAT])     # feat_future stoch part
    FFD = dout("ffd", [h_steps, BC, DETER])     # feat_future deter part

    with tile.TileContext(nc) as tc:
        with (
            tc.tile_pool(name="res", bufs=1) as res,        # resident constants
            tc.tile_pool(name="carry", bufs=2) as carry,    # stochT/deterT ping-pong
            tc.tile_pool(name="acts", bufs=1) as acts,      # per-step activation tensors
            tc.tile_pool(name="tmp", bufs=2) as tmp,        # small elementwise temps
            tc.tile_pool(name="wstream", bufs=1) as wst,    # streamed weights (bufs via tile())
            tc.tile_pool(name="pmm", bufs=4, space="PSUM") as pmm,
            tc.tile_pool(name="pwide", bufs=2, space="PSUM") as pwide,
            tc.tile_pool(name="ptr", bufs=2, space="PSUM") as ptr,
        ):
            # ---------------- setup: resident tiles ----------------
            ident = res.tile([128, 128], dt.float32, name="ident")
            make_identity(nc, ident[:])

            def load_res(name, dram, shape):
                tl = res.tile(list(shape), dt.float32, name=name)
                nc.sync.dma_start(tl[:], dram[tuple(slice(0, s) for s in shape)])
                return tl

            wim = load_res("wim_t", WIM, [128, 2048])
            cbt = load_res("cbt_t", CBT, [128, CODES])
            cb = load_res("cb_t", CB, [128, 512])
            cnorm = load_res("cnorm_t", CNORM, [128, CODES])
            w2 = load_res("w2_t", W2, [128, 2048])
            w3 = load_res("w3_t", W3, [128, 48])
            bim = load_res("bim_t", BIM, [128, 1])
            b1 = load_res("b1_t", B1, [128, 4])
            b2 = load_res("b2_t", B2, [128, 4])
            b3 = load_res("b3_t", B3, [ACT_DIM, 1])
            bin_t = {p: load_res(f"bin_t{p}", BIN[p], [128, 8]) for p in ("lam", "dyn")}
            bgru_t = {p: load_res(f"bgru_t{p}", BGRU[p], [128, 24]) for p in ("lam", "dyn")}
            bout_t = {p: load_res(f"bout_t{p}", BOUT[p], [128, 8]) for p in ("lam", "dyn")}
            blogit_t = {p: load_res(f"blogit_t{p}", BLOGIT[p], [128, SFLAT]) for p in ("lam", "dyn")}

            # initial carry
            stochT = carry.tile([128, 8 * BC], dt.float32, name="stochT0", tag="stochT")
            deterT = carry.tile([128, 8 * BC], dt.float32, name="deterT0", tag="deterT")
            nc.sync.dma_start(stochT[:], SSTOCHT[:, :])
            nc.sync.dma_start(deterT[:], SDETERT[:, :])

            # t=0 state outputs straight from DRAM inputs
            nc.sync.dma_start(OSTOCH[0, :, :], SSTOCH[:, :])
            nc.sync.dma_start(ODETER[0, :, :], SDETER[:, :])

            # rhs block helpers: feature-major tensors are [128, nblk*BC] tiles,
            # block k at cols [k*BC, (k+1)*BC)
            def blk(tl, k):
                return tl[:, k * BC:(k + 1) * BC]

            def elu1(dst_tile, n, psum, bias_col, t, p, nm):
                """dst block n <- elu(psum + bias) + 1 = relu(x+b) + exp(min(x+b,0))"""
                m = tmp.tile([128, BC], dt.float32, name=f"m{nm}", tag="elu_m")
                nc.vector.tensor_scalar(m[:], psum[:], bias_col, 0.0, OP.add, OP.min)
                e = tmp.tile([128, BC], dt.float32, name=f"e{nm}", tag="elu_e")
                nc.scalar.activation(e[:], m[:], AF.Exp)
                r = tmp.tile([128, BC], dt.float32, name=f"r{nm}", tag="elu_r")
                nc.scalar.activation(r[:], psum[:], AF.Relu, bias=bias_col)
                nc.vector.tensor_tensor(blk(dst_tile, n), r[:], e[:], OP.add)

            def transpose_to(dst_ap, src_ap, nm, use_act, kdim=128):
                """dst_ap (SBUF) <- transpose of src_ap [kdim,128] via PE + copy"""
                pt = ptr.tile([128, 128], dt.float32, name=f"pt{nm}", tag="ptr")
                nc.tensor.transpose(pt[0:src_ap.shape[1], 0:kdim], src_ap, ident[0:kdim, 0:kdim])
                if use_act:
                    nc.scalar.activation(dst_ap, pt[0:src_ap.shape[1], 0:kdim], AF.Copy)
                else:
                    nc.vector.tensor_copy(dst_ap, pt[0:src_ap.shape[1], 0:kdim])

            # ---------------- the imagination steps ----------------
            for t in range(h_steps):
                nm_t = f"_{t}"

                # ---- z = feat @ W_im + b_im   (feature-major, K=16 tiles) ----
                pz = pmm.tile([128, BC], dt.float32, name=f"pz{nm_t}", tag="pmm")
                for k in range(16):
                    rhs = blk(stochT, k) if k < 8 else blk(deterT, k - 8)
                    nc.tensor.matmul(pz[:], wim[:, k * 128:(k + 1) * 128], rhs,
                                     start=(k == 0), stop=(k == 15))
                zT = acts.tile([128, BC], dt.float32, name=f"zT{nm_t}", tag="zT")
                nc.vector.tensor_scalar(zT[:], pz[:], bim[:], None, OP.add)

                # ---- VQ: dist per batch-tile, argmin one-hot, transpose ----
                oh512T = acts.tile([128, 4 * BC], dt.float32, name=f"oh512T{nm_t}", tag="oh512T")
                for i in range(NB):
                    pd = pwide.tile([128, CODES], dt.float32, name=f"pd{nm_t}_{i}", tag="pwide")
                    nc.tensor.matmul(pd[:], zT[:, i * 128:(i + 1) * 128], cbt[:],
                                     start=True, stop=True)
                    dist = tmp.tile([128, CODES], dt.float32, name=f"dist{nm_t}_{i}", tag="dist")
                    nc.vector.scalar_tensor_tensor(dist[:], pd[:], -2.0, cnorm[:], OP.mult, OP.add)
                    dmin = tmp.tile([128, 1], dt.float32, name=f"dmin{nm_t}_{i}", tag="dmin")
                    nc.vector.tensor_reduce(dmin[:], dist[:], mybir.AxisListType.X, OP.min)
                    oh = tmp.tile([128, CODES], dt.float32, name=f"oh5{nm_t}_{i}", tag="oh512b")
                    nc.vector.tensor_scalar(oh[:], dist[:], dmin[:], None, OP.is_equal)
                    for c in range(4):
                        transpose_to(oh512T[:, c * BC + i * 128: c * BC + (i + 1) * 128],
                                     oh[:, c * 128:(c + 1) * 128],
                                     f"o{nm_t}_{i}_{c}", use_act=(c % 2 == 0))

                # quantT = codebook^T @ onehot  (K = 4 code tiles)
                pq = pmm.tile([128, BC], dt.float32, name=f"pq{nm_t}", tag="pmm")
                for c in range(4):
                    nc.tensor.matmul(pq[:], cb[:, c * 128:(c + 1) * 128],
                                     oh512T[:, c * BC:(c + 1) * BC],
                                     start=(c == 0), stop=(c == 3))
                quantT = acts.tile([128, BC], dt.float32, name=f"quantT{nm_t}", tag="quantT")
                nc.vector.tensor_copy(quantT[:], pq[:])

                # quant batch-major -> feats[t][:, 2048:2176]
                for i in range(NB):
                    qb = tmp.tile([128, LAT], dt.float32, name=f"qb{nm_t}_{i}", tag="qb")
                    transpose_to(qb[:], quantT[:, i * 128:(i + 1) * 128], f"qb{nm_t}_{i}",
                                 use_act=(i == 0))
                    nc.sync.dma_start(FQ[t, i * 128:(i + 1) * 128, :], qb[:])

                # ---- RSSM branch (shared for lam / dyn) ----
                def branch(p, act_tile):
                    nm = f"{nm_t}{p}"
                    # x_in = elu(concat(stoch, act) @ Win + bin) + 1
                    vT = acts.tile([128, 8 * BC], dt.float32, name=f"vT{nm}", tag="big8", bufs=2)
                    for n in range(8):
                        wch = wst.tile([128, 9 * 128], dt.float32, name=f"win{nm}_{n}",
                                       tag="win", bufs=2)
                        nc.sync.dma_start(wch[:], WIN[p][:, n * 1152:(n + 1) * 1152])
                        px = pmm.tile([128, BC], dt.float32, name=f"px{nm}_{n}", tag="pmm")
                        for k in range(9):
                            rhs = blk(stochT, k) if k < 8 else act_tile[:, 0:BC]
                            nc.tensor.matmul(px[:], wch[:, k * 128:(k + 1) * 128], rhs,
                                             start=(k == 0), stop=(k == 8))
                        elu1(vT, n, px, bin_t[p][:, n:n + 1], t, p, f"x{nm}_{n}")

                    # GRU: parts = concat(v, deter) @ Wgru + bgru_adj
                    if p == "lam":
                        ndet = acts.tile([128, 8 * BC], dt.float32, name=f"fdet{nm}", tag="big8", bufs=2)
                    else:
                        ndet = carry.tile([128, 8 * BC], dt.float32, name=f"ndet{nm}", tag="deterT")
                    for f in range(8):
                        pr = pmm.tile([128, BC], dt.float32, name=f"pr{nm}_{f}", tag="pmm")
                        pc = pmm.tile([128, BC], dt.float32, name=f"pc{nm}_{f}", tag="pmm")
                        pu = pmm.tile([128, BC], dt.float32, name=f"pu{nm}_{f}", tag="pmm")
                        gps = [pr, pc, pu]
                        for q in range(4):
                            gch = wst.tile([128, 4 * 384], dt.float32, name=f"wgru{nm}_{f}_{q}",
                                           tag="wgru", bufs=2)
                            nc.sync.dma_start(gch[:], WGRU[p][:, f * 6144 + q * 1536:
                                                              f * 6144 + (q + 1) * 1536])
                            for kk in range(4):
                                k = q * 4 + kk
                                rhs = blk(vT, k) if k < 8 else blk(deterT, k - 8)
                                for g in range(3):
                                    nc.tensor.matmul(gps[g][:],
                                                     gch[:, kk * 384 + g * 128: kk * 384 + (g + 1) * 128],
                                                     rhs, start=(k == 0), stop=(k == 15))
                        bg = bgru_t[p]
                        r_sb = tmp.tile([128, BC], dt.float32, name=f"gr{nm}_{f}", tag="g_r")
                        nc.scalar.activation(r_sb[:], pr[:], AF.Sigmoid, bias=bg[:, f:f + 1])
                        t_sb = tmp.tile([128, BC], dt.float32, name=f"gt{nm}_{f}", tag="g_t")
                        nc.vector.scalar_tensor_tensor(t_sb[:], pc[:], bg[:, 8 + f:9 + f], r_sb[:],
                                                       OP.add, OP.mult)
                        c_sb = tmp.tile([128, BC], dt.float32, name=f"gc{nm}_{f}", tag="g_c")
                        nc.scalar.activation(c_sb[:], t_sb[:], AF.Tanh)
                        u_sb = tmp.tile([128, BC], dt.float32, name=f"gu{nm}_{f}", tag="g_u")
                        nc.scalar.activation(u_sb[:], pu[:], AF.Sigmoid, bias=bg[:, 16 + f:17 + f])
                        s_sb = tmp.tile([128, BC], dt.float32, name=f"gs{nm}_{f}", tag="g_s")
                        nc.vector.tensor_tensor(s_sb[:], c_sb[:], blk(deterT, f), OP.subtract)
                        w_sb = tmp.tile([128, BC], dt.float32, name=f"gw{nm}_{f}", tag="g_w")
                        nc.vector.tensor_tensor(w_sb[:], s_sb[:], u_sb[:], OP.mult)
                        nc.vector.tensor_tensor(blk(ndet, f), w_sb[:], blk(deterT, f), OP.add)

                    # y' = elu(ndet @ Wout + bout) + 1
                    yT = acts.tile([128, 8 * BC], dt.float32, name=f"yT{nm}", tag="big8", bufs=2)
                    for n in range(8):
                        och = wst.tile([128, 8 * 128], dt.float32, name=f"wout{nm}_{n}",
                                       tag="wout", bufs=2)
                        nc.sync.dma_start(och[:], WOUT[p][:, n * 1024:(n + 1) * 1024])
                        py = pmm.tile([128, BC], dt.float32, name=f"py{nm}_{n}", tag="pmm")
                        for k in range(8):
                            nc.tensor.matmul(py[:], och[:, k * 128:(k + 1) * 128], blk(ndet, k),
                                             start=(k == 0), stop=(k == 7))
                        elu1(yT, n, py, bout_t[p][:, n:n + 1], t, p, f"y{nm}_{n}")

                    # logits (batch-major, option A): lhsT = yT slices, rhs = Wlogit
                    lg = [acts.tile([128, SFLAT], dt.float32, name=f"lg{nm}_{i}", tag=f"lg{i}")
                          for i in range(NB)]
                    for hh in range(2):
                        pl = [pwide.tile([128, 512], dt.float32, name=f"pl{nm}_{hh}_{i}",
                                         tag="pwide") for i in range(NB)]
                        for k2 in range(4):
                            lch = wst.tile([128, 2 * 512], dt.float32, name=f"wlog{nm}_{hh}_{k2}",
                                           tag="wlogit", bufs=2)
                            nc.sync.dma_start(lch[:], WLOGIT[p][:, hh * 4096 + k2 * 1024:
                                                                hh * 4096 + (k2 + 1) * 1024])
                            for kk in range(2):
                                k = k2 * 2 + kk
                                for i in range(NB):
                                    nc.tensor.matmul(pl[i][:],
                                                     yT[:, k * BC + i * 128: k * BC + (i + 1) * 128],
                                                     lch[:, kk * 512:(kk + 1) * 512],
                                                     start=(k == 0), stop=(k == 7))
                        for i in range(NB):
                            nc.vector.tensor_tensor(lg[i][:, hh * 512:(hh + 1) * 512], pl[i][:],
                                                    blogit_t[p][:, hh * 512:(hh + 1) * 512], OP.add)
                    ohb = []
                    for i in range(NB):
                        mx = tmp.tile([128, STOCH], dt.float32, name=f"mx{nm}_{i}", tag="mx")
                        lg3 = lg[i][:].rearrange("p (g d) -> p g d", d=DISC)
                        nc.vector.tensor_reduce(mx[:], lg3, mybir.AxisListType.X, OP.max)
                        ohi = acts.tile([128, SFLAT], dt.float32, name=f"ohb{nm}_{i}",
                                        tag=f"ohb_{i}")
                        nc.vector.tensor_tensor(ohi[:].rearrange("p (g d) -> p g d", d=DISC), lg3,
                                                mx[:, :, None].broadcast_to([128, STOCH, DISC]),
                                                OP.is_equal)
                        ohb.append(ohi)
                    return vT, ndet, yT, ohb

                # ======== LAM branch -> feat_future[t] ========
                vT_l, fdet, yT_l, ohb_l = branch("lam", quantT)
                for i in range(NB):
                    nc.sync.dma_start(FFS[t, i * 128:(i + 1) * 128, :], ohb_l[i][:])
                for i in range(NB):
                    db = tmp.tile([128, DETER], dt.float32, name=f"dbl{nm_t}_{i}", tag=f"db{i}", bufs=1)
                    for f in range(8):
                        transpose_to(db[:, f * 128:(f + 1) * 128],
                                     fdet[:, f * BC + i * 128: f * BC + (i + 1) * 128],
                                     f"dl{nm_t}_{i}_{f}", use_act=(f % 2 == 0))
                    nc.sync.dma_start(FFD[t, i * 128:(i + 1) * 128, :], db[:])

                # ======== actor: action = tanh(MLP(concat(feat_ori, quant))) ========
                h1T = acts.tile([128, 4 * BC], dt.float32, name=f"h1T{nm_t}", tag="h1T")
                for n in range(4):
                    wch = wst.tile([128, 17 * 128], dt.float32, name=f"w1{nm_t}_{n}",
                                   tag="w1", bufs=1)
                    nc.sync.dma_start(wch[:], W1[:, n * 2176:(n + 1) * 2176])
                    ph = pmm.tile([128, BC], dt.float32, name=f"ph1{nm_t}_{n}", tag="pmm")
                    for k in range(17):
                        rhs = blk(stochT, k) if k < 8 else (
                            blk(deterT, k - 8) if k < 16 else quantT[:, 0:BC])
                        nc.tensor.matmul(ph[:], wch[:, k * 128:(k + 1) * 128], rhs,
                                         start=(k == 0), stop=(k == 16))
                    elu1(h1T, n, ph, b1[:, n:n + 1], t, "a", f"h1{nm_t}_{n}")
                h2T = acts.tile([128, 4 * BC], dt.float32, name=f"h2T{nm_t}", tag="h2T")
                for n in range(4):
                    ph = pmm.tile([128, BC], dt.float32, name=f"ph2{nm_t}_{n}", tag="pmm")
                    for k in range(4):
                        nc.tensor.matmul(ph[:], w2[:, n * 512 + k * 128: n * 512 + (k + 1) * 128],
                                         blk(h1T, k), start=(k == 0), stop=(k == 3))
                    elu1(h2T, n, ph, b2[:, n:n + 1], t, "a", f"h2{nm_t}_{n}")
                pa = pmm.tile([ACT_DIM, BC], dt.float32, name=f"pa{nm_t}", tag="pmm")
                for k in range(4):
                    nc.tensor.matmul(pa[:], w3[:, k * ACT_DIM:(k + 1) * ACT_DIM], blk(h2T, k),
                                     start=(k == 0), stop=(k == 3))
                actT = acts.tile([128, BC], dt.float32, name=f"actT{nm_t}", tag="actT")
                nc.gpsimd.memset(actT[:], 0.0)
                nc.scalar.activation(actT[0:ACT_DIM, :], pa[:], AF.Tanh, bias=b3[:])
                for i in range(NB):
                    ab = tmp.tile([128, ACT_DIM], dt.float32, name=f"ab{nm_t}_{i}", tag="ab")
                    transpose_to(ab[:], actT[0:ACT_DIM, i * 128:(i + 1) * 128],
                                 f"ab{nm_t}_{i}", use_act=(i == 0), kdim=ACT_DIM)
                    nc.sync.dma_start(ACTIONS[t, i * 128:(i + 1) * 128, :], ab[:])

                # ======== DYN branch -> next state (skipped at final step) ========
                if t < h_steps - 1:
                    vT_d, ndet, yT_d, ohb_d = branch("dyn", actT)
                    # next-state batch-major outputs
                    for i in range(NB):
                        sl = slice(i * 128, (i + 1) * 128)
                        nc.sync.dma_start(OSTOCH[t + 1, sl, :], ohb_d[i][:])
                        db = tmp.tile([128, DETER], dt.float32, name=f"dbd{nm_t}_{i}", tag=f"db{i}", bufs=1)
                        for f in range(8):
                            transpose_to(db[:, f * 128:(f + 1) * 128],
                                         ndet[:, f * BC + i * 128: f * BC + (i + 1) * 128],
                                         f"dd{nm_t}_{i}_{f}", use_act=(f % 2 == 1))
                        nc.sync.dma_start(ODETER[t + 1, sl, :], db[:])
                    # next stochT (feature-major) from one-hots
                    nstoch = carry.tile([128, 8 * BC], dt.float32, name=f"nst{nm_t}", tag="stochT")
                    for f in range(8):
                        for i in range(NB):
                            transpose_to(nstoch[:, f * BC + i * 128: f * BC + (i + 1) * 128],
                                         ohb_d[i][:, f * 128:(f + 1) * 128],
                                         f"st{nm_t}_{i}_{f}", use_act=((f + i) % 2 == 0))
                    stochT, deterT = nstoch, ndet

    nc.compile()
    names = dict(
        outs=["actions", "ostoch", "odeter", "fq", "ffs", "ffd"],
    )
    return nc, names


def _prep_shared(d):
    """Host-side packing of weights/biases shared by all cores."""
    f = _f32
    w_im = f(d["W_im"]); b_im = f(d["b_im"]); cbk = f(d["codebook"])
    out = {}
    out["wim"] = w_im.reshape(16, 128, 128).transpose(1, 0, 2).reshape(128, 2048).copy()
    out["cbt"] = cbk.T.copy()
    out["cb"] = cbk.reshape(4, 128, 128).transpose(1, 0, 2).reshape(128, 512).copy()
    out["cnorm"] = np.tile((cbk * cbk).sum(1)[None, :], (128, 1)).astype(np.float32)
    out["bim"] = b_im[:, None].copy()
    for p, adim in (("lam", LAT), ("dyn", ACT_DIM)):
        win = f(d[p + "_Win"]); bin_ = f(d[p + "_bin"])
        wgru = f(d[p + "_Wgru"]); bgru = f(d[p + "_bgru"])
        wout = f(d[p + "_Wout"]); bout = f(d[p + "_bout"])
        wlog = f(d[p + "_Wlogit"]); blog = f(d[p + "_blogit"])
        win_pad = np.zeros((1152, 1024), np.float32)
        win_pad[: win.shape[0]] = win
        out[f"win_{p}"] = (win_pad.reshape(9, 128, 8, 128)
                           .transpose(2, 0, 1, 3)    # [n, k, p, c]
                           .transpose(2, 0, 1, 3)    # [p, n, k, c]
                           .reshape(128, 8 * 9 * 128).copy())
        bgru_adj = bgru - wgru[:1024].sum(0)
        bgru_adj = bgru_adj.copy()
        bgru_adj[2048:] -= 1.0    # update gate's  sigmoid(u - 1)
        out[f"wgru_{p}"] = (wgru.reshape(16, 128, 3, 8, 128)
                            .transpose(1, 3, 0, 2, 4)   # [p, f, k, g, c]
                            .reshape(128, 8 * 16 * 3 * 128).copy())
        out[f"wout_{p}"] = (wout.reshape(8, 128, 8, 128)
                            .transpose(1, 2, 0, 3)      # [p, n, k, c]
                            .reshape(128, 8 * 8 * 128).copy())
        out[f"wlogit_{p}"] = (wlog.reshape(8, 128, 2, 512)
                              .transpose(1, 2, 0, 3)    # [p, h, k, c]
                              .reshape(128, 2 * 8 * 512).copy())
        blog_adj = blog - wlog.sum(0)
        out[f"bin_{p}"] = bin_.reshape(8, 128).T.copy()
        out[f"bgru_{p}"] = bgru_adj.reshape(3, 8, 128).transpose(2, 0, 1).reshape(128, 24).copy()
        out[f"bout_{p}"] = bout.reshape(8, 128).T.copy()
        out[f"blogit_{p}"] = np.tile(blog_adj[None, :], (128, 1)).astype(np.float32)
    w1 = f(d["act_W1"]); w2_ = f(d["act_W2"]); w3_ = f(d["act_W3"])
    out["w1"] = (w1.reshape(17, 128, 4, 128).transpose(1, 2, 0, 3)
                 .reshape(128, 4 * 17 * 128).copy())
    out["w2"] = (w2_.reshape(4, 128, 4, 128).transpose(1, 2, 0, 3)
                 .reshape(128, 4 * 4 * 128).copy())
    out["w3"] = w3_.reshape(4, 128, ACT_DIM).transpose(1, 0, 2).reshape(128, 48).copy()
    out["b1"] = f(d["act_b1"]).reshape(4, 128).T.copy()
    out["b2"] = (f(d["act_b2"]) - w2_.sum(0)).reshape(4, 128).T.copy()
    out["b3"] = (f(d["act_b3"]) - w3_.sum(0))[:, None].copy()
    return out


def _prep_core(d, core):
    rows = slice(core * BC, (core + 1) * BC)
    ss = _f32(d["start_stoch"])[rows]
    sd = _f32(d["start_deter"])[rows]
    return {
        "sstocht": ss.T.reshape(8, 128, BC).transpose(1, 0, 2).reshape(128, 8 * BC).copy(),
        "sdetert": sd.T.reshape(8, 128, BC).transpose(1, 0, 2).reshape(128, 8 * BC).copy(),
        "sstoch": ss.copy(),
        "sdeter": sd.copy(),
    }


def _in_maps(inputs):
    shared = _prep_shared(inputs)
    return [dict(shared, **_prep_core(inputs, core)) for core in range(NCORES)]


def _assemble(parts):
    """parts: dict name -> full [H, B, D] array; returns reference-order tuple."""
    feat_ori = np.concatenate([parts["ostoch"], parts["odeter"]], axis=-1)
    feats = np.concatenate([feat_ori, parts["fq"]], axis=-1)
    feat_future = np.concatenate([parts["ffs"], parts["ffd"]], axis=-1)
    return (feats, parts["actions"], parts["ostoch"], parts["odeter"],
            feat_ori, feat_future)


def kernel(h_steps=H, **inputs):
    key = h_steps
    if key not in _cached:
        _cached[key] = build(h_steps)
    nc, names = _cached[key]
    res = bass_utils.run_bass_kernel_spmd(nc, _in_maps(inputs),
                                          core_ids=list(range(NCORES)))
    parts = {
        nm: np.concatenate([res.results[c][nm] for c in range(NCORES)], axis=1)
        for nm in names["outs"]
    }
    return _assemble(parts)


def timed_run(inputs, h_steps=H, iters=3):
    """Run on 8 cores with device-staged inputs; return (outputs, best wall ns)."""
    import time
    import jax
    from jax.experimental.shard_map import shard_map
    from jax.sharding import Mesh, NamedSharding, PartitionSpec
    from concourse import bass2jax, mybir as _mybir

    key = h_steps
    if key not in _cached:
        _cached[key] = build(h_steps)
    nc, names = _cached[key]
    bass2jax.install_neuronx_cc_hook()
    in_maps = _in_maps(inputs)

    partition_name = nc.partition_id_tensor.name if nc.partition_id_tensor else None
    in_names, out_names, out_avals, zero_outs = [], [], [], []
    for alloc in nc.m.functions[0].allocations:
        if not isinstance(alloc, _mybir.MemoryLocationSet):
            continue
        name = alloc.memorylocations[0].name
        if alloc.kind == "ExternalInput":
            if name != partition_name:
                in_names.append(name)
        elif alloc.kind == "ExternalOutput":
            shape = tuple(alloc.tensor_shape)
            np_dt = _mybir.dt.np(alloc.dtype)
            out_names.append(name)
            out_avals.append(jax.core.ShapedArray(shape, np_dt))
            zero_outs.append(np.zeros(shape, np_dt))
    n_params, n_outs = len(in_names), len(out_names)
    all_in = in_names + out_names + ([partition_name] if partition_name else [])

    def _body(*args):
        operands = list(args)
        if partition_name is not None:
            operands.append(bass2jax.partition_id_tensor())
        return tuple(bass2jax._bass_exec_p.bind(
            *operands, out_avals=tuple(out_avals), in_names=tuple(all_in),
            out_names=tuple(out_names), lowering_input_output_aliases=(),
            sim_require_finite=True, sim_require_nnan=True, nc=nc))

    devices = jax.devices()[:NCORES]
    mesh = Mesh(np.asarray(devices), ("core",))
    sharded = jax.jit(
        shard_map(_body, mesh=mesh, in_specs=(PartitionSpec("core"),) * (n_params + n_outs),
                  out_specs=(PartitionSpec("core"),) * n_outs, check_rep=False),
        donate_argnums=tuple(range(n_params, n_params + n_outs)), keep_unused=True)

    sh = NamedSharding(mesh, PartitionSpec("core"))
    concat_in = [
        jax.device_put(np.concatenate([np.asarray(in_maps[c][nm]) for c in range(NCORES)], 0), sh)
        for nm in in_names
    ]
    best = None
    out_arrs = None
    for _ in range(iters):
        zeros = [jax.device_put(np.zeros((NCORES * z.shape[0], *z.shape[1:]), z.dtype), sh)
                 for z in zero_outs]
        jax.block_until_ready(zeros)
        jax.block_until_ready(concat_in)
        t0 = time.perf_counter()
        out_arrs = sharded(*concat_in, *zeros)
        jax.block_until_ready(out_arrs)
        dtns = (time.perf_counter() - t0) * 1e9
        print(f"  iter wall: {dtns/1e6:.2f} ms")
        best = dtns if best is None else min(best, dtns)
    parts = {}
    for nm in names["outs"]:
        i = out_names.index(nm)
        a = np.asarray(out_arrs[i]).reshape(NCORES, *out_avals[i].shape)
        parts[nm] = np.concatenate([a[c] for c in range(NCORES)], axis=1)
    return _assemble(parts), best
